# revision 25
# baseline (speedup 1.0000x reference)
"""CQAttention (QANet context-query attention) Trainium2 Bass kernel.

Full-input contract: kernel(C, Q, cmask, qmask, w) -> (B, 4D, LC) f32.
Shards batch B=16 across 8 NeuronCores (2 examples/core), runs one SPMD
Bass/Tile program, gathers results.

Math (per example, d=512, Lc=2048, Lq=512):
  S = Cb@w1 [i] + Qb@w2 [j] + (Cb*w3)@Qb^T          (Lc, Lq)
  S1 = softmax_j(S), S2 = softmax_i(S)
  A = S1@Qb ; Bt = S1@S2^T@Cb
  out = concat([Cb, A, Cb*A, Cb*Bt], feat).T        (4d, Lc)

fp8 version of the bf16 pipeline (same phase skeleton):
  - all big matmul operands are fp8e4 (C~N(0,1), exp(S)<~150<448: in
    range); E2/A/B/colsum matmuls use MatmulPerfMode.DoubleRow (two
    128-contractions per instruction at 0.5 cycles/col = 4x bf16 rate).
    T2 keeps per-ki streaming so it stays at 1 col/cycle.
  - softmax weights are NEVER materialized in fp8 (a normalized weight
    ~1/512 sits at e4m3's denormal step -> ~50% quantization error).
    E1T stays the raw exp(S) transpose; the S1 normalizer recrep[i]
    (replicated across partitions by the colsum matmul trick) is
    applied at the f32 output muls instead:
      o2 = aps_raw * recrep ; o3 = o2 * C ; o4 = bps_raw * (C*recrep)
  - exp(c2[j]) (the Qb@w2 softmax-axis bias for S1) is folded for free
    into per-partition-scaled fp8 casts: Qbt' = Qb^T * ec2, T2s' =
    T2norm * ec2, and an ec2mat stationary replaces all-ones in the
    colsum matmul. E1T copies are then scale-free and batch to ONE
    [128,(4),128] activation per ki (64 -> 16 ACT instructions/example).
  - engine split to keep every engine under the ~120us DMA floor:
    PE matmuls/transposes; ACT exp, E1T/cbt/Qbt/T2s casts; DVE CtR
    cast, o2/o4 muls (PSUM reads); POOL (was idle) o3 + C*recrep muls
    (SBUF-only operands).
  - emission order software-pipelines the two examples as before.
"""

import numpy as np

import concourse.bass as bass
import concourse.tile as tile
from concourse import bacc, mybir
from concourse.bass_utils import run_bass_kernel_spmd
from concourse.masks import make_identity

B, D, LC, LQ = 16, 512, 2048, 512
NCORES = 8
BL = B // NCORES  # examples per core
KD = D // 128  # 4 d-chunks
KJ = LQ // 128  # 4 j-chunks
NI = LC // 512  # 4 i column-chunks
MI = LC // 128  # 16 i partition-chunks

F32 = mybir.dt.float32
F32R = mybir.dt.float32r
BF16 = mybir.dt.bfloat16
F8 = mybir.dt.float8e4
DR = mybir.MatmulPerfMode.DoubleRow
EXP = mybir.ActivationFunctionType.Exp
COPY = mybir.ActivationFunctionType.Copy
IDENT = mybir.ActivationFunctionType.Identity
MUL = mybir.AluOpType.mult
ADD = mybir.AluOpType.add


class Ctx:
    pass


def _pools(tc, ctx):
    P = Ctx()
    P.const = ctx.enter_context(tc.tile_pool(name="const", bufs=1))
    P.cstage = ctx.enter_context(tc.tile_pool(name="cstage", bufs=2))
    P.qt = ctx.enter_context(tc.tile_pool(name="qt", bufs=1))
    P.big = ctx.enter_context(tc.tile_pool(name="big", bufs=1))
    P.mid = ctx.enter_context(tc.tile_pool(name="mid", bufs=1))
    P.stream = ctx.enter_context(tc.tile_pool(name="stream", bufs=1))
    P.ost = ctx.enter_context(tc.tile_pool(name="ost", bufs=3))
    P.psum = ctx.enter_context(tc.tile_pool(name="psum", space="PSUM", bufs=8))
    return P


def _phase_A_loads(nc, P, T, Cd, Qd, Od, b):
    """Input DMAs: one for Q, four column-quarter DMAs for C (each
    covers all four d-chunks via a rearranged AP and is aligned with
    the e1 column block that consumes it). Few, large issues so the
    DGE ring never stalls."""
    T.Qt = P.qt.tile([128, KD, LQ], F32, tag="qt", name=f"qt{b}")
    for h in range(2):
        nc.sync.dma_start(
            out=T.Qt[:, 2 * h : 2 * h + 2, :],
            in_=Qd[b].rearrange("(m p) f -> p m f", p=128)[:, 2 * h : 2 * h + 2, :],
        )
    T.cst = P.cstage.tile(
        [128, KD, LC], F32, tag="cstage", bufs=2, name=f"cst{b}"
    )
    for q in range(4):
        qsl = slice(q * 512, (q + 1) * 512)
        nc.sync.dma_start(
            out=T.cst[:, :, qsl],
            in_=Cd[b].rearrange("(m p) f -> p m f", p=128)[:, :, qsl],
        )


def _o1_writes(nc, T, Od, b):
    """out rows 0..D-1 are exactly C[b]; placed inside the CD window
    where output DMA is otherwise idle."""
    nc.sync.dma_start(
        out=Od[b, 0:D, :].rearrange("(m p) f -> p m f", p=128), in_=T.cst
    )


def _phase_A_body(nc, P, K, T, b):
    """fp8 operand prep (Qmod/CtR), Q transpose -> Qbt' (ec2-folded),
    c2 bias columns + ec2mat colsum stationary."""
    psum = P.psum
    QtR = P.qt.tile([128, KD, LQ], BF16, tag="qtr", name=f"qtr{b}")
    T.Qmod = P.mid.tile([128, KD, LQ], F8, tag="qmod", name=f"qmod{b}")
    T.CtR = P.mid.tile([128, KD, LC], F8, tag="cbig", bufs=2, name=f"ctr{b}")
    # wsb cols: 0-3 w1, 4-7 w2, 8-11 w3. Casts live on DVE: at the
    # example boundary ACT is the pacer while DVE is idle.
    for k in range(KD):
        nc.vector.tensor_copy(QtR[:, k, :], T.Qt[:, k, :])
    for q in range(4):
        qsl = slice(q * 512, (q + 1) * 512)
        for k in range(KD):
            nc.vector.tensor_copy(T.CtR[:, k, qsl], T.cst[:, k, qsl])
    qps = [
        psum.tile([128, D], BF16, tag="ps", name=f"qps{b}_{c}") for c in range(KJ)
    ]
    for a in range(KD):
        for c in range(KJ):
            nc.tensor.transpose(
                qps[c][:, a * 128 : (a + 1) * 128],
                QtR[:, a, c * 128 : (c + 1) * 128],
                K.identR,
            )
    # c2[j] = Q^T w2, computed as a row then transposed to per-partition
    # columns
    c2row_ps = psum.tile([1, LQ], F32, tag="ps", name=f"c2rp{b}")
    for kd in range(KD):
        nc.tensor.matmul(
            c2row_ps, K.wsbR[:, 4 + kd : 5 + kd], QtR[:, kd, :],
            start=(kd == 0), stop=(kd == KD - 1),
        )
    c2row = P.stream.tile([1, LQ], F32, tag="c2row", name=f"c2r{b}")
    nc.scalar.copy(c2row, c2row_ps)
    c2ps = psum.tile([128, KJ], F32, tag="ps", name=f"c2ps{b}")
    for jm in range(KJ):
        nc.tensor.transpose(
            c2ps[:, jm : jm + 1],
            c2row[:, jm * 128 : (jm + 1) * 128],
            K.ident[:1, :1],
        )
    T.ec2col = P.mid.tile([128, KJ], F32, tag="c2col", name=f"c2col{b}")
    nc.scalar.activation(T.ec2col, c2ps, EXP)
    # colsum stationary: column f of chunk c is ec2col[:, c] for all f
    T.ec2mat = P.mid.tile([128, KJ, 128], F8, tag="c2mat", name=f"c2mat{b}")
    for c in range(KJ):
        nc.scalar.activation(
            T.ec2mat[:, c, :], K.ones_matb, COPY, scale=T.ec2col[:, c : c + 1]
        )
    for k in range(KD):
        nc.vector.tensor_scalar(
            T.Qmod[:, k, :], T.Qt[:, k, :],
            K.wsb[:, 8 + k : 9 + k], K.wsb[:, k : k + 1], MUL, ADD,
        )
    # Qbt' = Qb^T * ec2 (fp8): per-partition ec2 scale folded into cast
    for c in range(KJ):
        nc.vector.tensor_scalar_mul(
            T.Qbt[:, c, :], qps[c], T.ec2col[:, c : c + 1]
        )


MP = MI // 2  # 8 pair-steps of two 128-row i-chunks


def _phase_CD(nc, P, K, T, Od, b, inject=None):
    """Stream E2 row-chunks in ki-PAIRS so the T2/ssum contractions ride
    fp8 DoubleRow too (2 k-tiles per instruction at 2x bf16 math rate).
    Per pair: cbt transposes (8) + 1 packing copy, 2x(E2 DR matmuls +
    exp), then the trailing consume of the previous pair: ejt
    transposes (8) + 1 batched E1T copy + 1 DR ssum + 4 DR T2 matmuls.
    colsum blocks and the A^T/o2/o3 consumers of finished E1T column
    blocks interleave at pair boundaries so output DMA streams from
    early in the phase. `inject` maps pair-step -> closure."""
    psum = P.psum
    t2ps = [
        psum.tile([128, D], F32, tag="ps", name=f"t2ps{b}_{m}") for m in range(KJ)
    ]
    ssps = psum.tile([1, LQ], F32, tag="ps", name=f"ssps{b}")
    T.t2ps, T.ssps = t2ps, ssps
    e2prs, cbt_prs = {}, {}
    inject = inject or {}
    T.recrep = {}

    def consume(kp):
        e2pr = e2prs.pop(kp)
        psl = slice(kp * 256, (kp + 1) * 256)
        # fp8 PE transposes must write with element step 2 (walrus rule);
        # the packing copy reads the stride-2 view
        ejt_ps = psum.tile([128, KJ, 256, 2], F8, tag="ps", name=f"ejt{b}_{kp}")
        for t in range(2):
            for kj in range(KJ):
                nc.tensor.transpose(
                    ejt_ps[:, kj, t * 128 : (t + 1) * 128, 0],
                    e2pr[:, t, kj * 128 : (kj + 1) * 128],
                    K.identF8,
                )
        nc.scalar.copy(T.E1T[:, :, psl], ejt_ps[:, :, :, 0])
        for t in range(2):
            nc.tensor.matmul(
                ssps, K.ones_col, e2pr[:, t, :],
                start=(kp == 0 and t == 0), stop=(kp == MP - 1 and t == 1),
            )
        cbt_pr = cbt_prs.pop(kp)
        for mj in range(KJ):
            nc.tensor.matmul(
                t2ps[mj], e2pr[:, :, mj * 128 : (mj + 1) * 128], cbt_pr,
                start=(kp == 0), stop=(kp == MP - 1), perf_mode=DR,
            )

    colA = {3: 0, 5: 1, 7: 2}
    colE = {4: 0, 6: 1}
    for kp in range(MP):
        if kp in inject:
            inject[kp]()
        if kp in colA:
            _colsum_produce(nc, P, K, T, b, colA[kp])
        if kp in colE:
            _E_aps(nc, P, K, T, Od, b, colE[kp])
        cbt_ps = psum.tile(
            [128, 2, KD, 128, 2], F8, tag="ps", name=f"cps{b}_{kp}"
        )
        for t in range(2):
            isl = slice((2 * kp + t) * 128, (2 * kp + t + 1) * 128)
            for kd in range(KD):
                nc.tensor.transpose(
                    cbt_ps[:, t, kd, :, 0], T.CtR[:, kd, isl], K.identF8
                )
        cbt_pr = P.stream.tile(
            [128, 2, KD, 128], F8, tag="cbt", bufs=3, name=f"cbt{b}_{kp}"
        )
        nc.scalar.copy(cbt_pr, cbt_ps[:, :, :, :, 0])
        cbt_prs[kp] = cbt_pr

        e2pr = P.stream.tile(
            [128, 2, LQ], F8, tag="e2", bufs=3, name=f"e2pr{b}_{kp}"
        )
        for t in range(2):
            isl = slice((2 * kp + t) * 128, (2 * kp + t + 1) * 128)
            e2ps = psum.tile([128, LQ], F32, tag="ps", name=f"e2ps{b}_{kp}_{t}")
            for h in range(2):
                nc.tensor.matmul(
                    e2ps, T.CtR[:, 2 * h : 2 * h + 2, isl],
                    T.Qmod[:, 2 * h : 2 * h + 2, :],
                    start=(h == 0), stop=(h == 1), perf_mode=DR,
                )
            nc.scalar.activation(e2pr[:, t, :], e2ps, EXP)
        e2prs[kp] = e2pr
        if kp > 0:
            consume(kp - 1)
    consume(MP - 1)
    _colsum_produce(nc, P, K, T, b, 3)
    _rec2_block(nc, P, K, T, b)
    # tail: A/B output phases for the last two column blocks + all of
    # Bt. Returned as closures so build() can spread them across the
    # NEXT example's CD loop (output DMA + POOL/DVE muls overlap the
    # next example's PE/ACT-heavy stream instead of bursting at the
    # end); for the last example they run back-to-back at the end.
    return [
        lambda: _E_aps(nc, P, K, T, Od, b, 2),
        lambda: _E_aps(nc, P, K, T, Od, b, 3),
        lambda: _E_bps(nc, P, K, T, Od, b, 0),
        lambda: _E_bps(nc, P, K, T, Od, b, 1),
        lambda: _E_bps(nc, P, K, T, Od, b, 2),
        lambda: _E_bps(nc, P, K, T, Od, b, 3),
    ]


def _colsum_produce(nc, P, K, T, b, ni):
    """ec2-weighted column sums of raw E1T, replicated across
    partitions by the ec2mat stationary, then approximate reciprocal
    straight off PSUM."""
    psum = P.psum
    nsl = slice(ni * 512, (ni + 1) * 512)
    csps = psum.tile([128, 512], F32, tag="ps", name=f"csps{b}_{ni}")
    for h in range(2):
        nc.tensor.matmul(
            csps, T.ec2mat[:, 2 * h : 2 * h + 2, :],
            T.E1T[:, 2 * h : 2 * h + 2, nsl],
            start=(h == 0), stop=(h == 1), perf_mode=DR,
        )
    T.recrep[ni] = P.stream.tile(
        [128, 512], F32, tag="recrep", bufs=6, name=f"rr{b}_{ni}"
    )
    nc.vector.reciprocal_approx_fast(out=T.recrep[ni], in_=csps)


def _rec2_block(nc, P, K, T, b):
    rec2row = P.stream.tile([1, LQ], F32, tag="rec2row", name=f"r2r{b}")
    nc.vector.reciprocal_approx_fast(out=rec2row, in_=T.ssps)
    rc_ps = P.psum.tile([128, KJ], F32, tag="ps", name=f"rcps{b}")
    for jm in range(KJ):
        nc.tensor.transpose(
            rc_ps[:, jm : jm + 1],
            rec2row[:, jm * 128 : (jm + 1) * 128],
            K.ident[:1, :1],
        )
    # comb = (1/ssum) * ec2, folded into the T2s cast scale
    comb = P.stream.tile([128, KJ], F32, tag="rec2col", name=f"r2c{b}")
    nc.vector.tensor_mul(comb, rc_ps, T.ec2col)
    for mj in range(KJ):
        nc.vector.tensor_scalar_mul(
            T.T2s[:, mj, :], T.t2ps[mj], comb[:, mj : mj + 1]
        )


def _E_aps(nc, P, K, T, Od, b, ni):
    """A^T and C*A^T for one 512-wide i column block (needs only the
    raw E1T block, recrep and Qbt', so it runs long before CD/T2).
    o2 = aps*recrep (DVE, PSUM read); o3 = o2*C split DVE/POOL."""
    psum = P.psum
    nsl = slice(ni * 512, (ni + 1) * 512)
    for md in range(4):
        msl = slice(md * 128, (md + 1) * 128)
        aps = psum.tile([128, 512], F32, tag="ps", name=f"aps{b}_{md}_{ni}")
        for h in range(2):
            nc.tensor.matmul(
                aps, T.Qbt[:, 2 * h : 2 * h + 2, msl],
                T.E1T[:, 2 * h : 2 * h + 2, nsl],
                start=(h == 0), stop=(h == 1), perf_mode=DR,
            )
        o2 = P.ost.tile([128, 512], F32, tag="o2", bufs=5, name=f"o2_{b}_{md}_{ni}")
        nc.vector.tensor_mul(o2, aps, T.recrep[ni])
        o3 = P.ost.tile([128, 512], F32, tag="o3", bufs=5, name=f"o3_{b}_{md}_{ni}")
        eng = nc.gpsimd if md % 2 else nc.vector
        eng.tensor_mul(o3, o2, T.cst[:, md, nsl])
        nc.sync.dma_start(
            out=Od[b, D + md * 128 : D + (md + 1) * 128, nsl], in_=o2
        )
        nc.sync.dma_start(
            out=Od[b, 2 * D + md * 128 : 2 * D + (md + 1) * 128, nsl], in_=o3
        )


def _E_bps(nc, P, K, T, Od, b, ni):
    """Bt^T and C*Bt^T for one column block; runs after CD/rec2 produce
    T2s'. crn = C*recrep (POOL) so o4 = bps*crn is one DVE mul."""
    psum = P.psum
    nsl = slice(ni * 512, (ni + 1) * 512)
    crn = P.ost.tile([128, KD, 512], F32, tag="crn", bufs=2, name=f"crn{b}_{ni}")
    for kd in range(KD):
        eng = nc.gpsimd if kd % 2 else nc.vector
        eng.tensor_mul(crn[:, kd, :], T.cst[:, kd, nsl], T.recrep[ni])
    for md in range(4):
        msl = slice(md * 128, (md + 1) * 128)
        bps = psum.tile([128, 512], F32, tag="ps", name=f"bps{b}_{md}_{ni}")
        for h in range(2):
            nc.tensor.matmul(
                bps, T.T2s[:, 2 * h : 2 * h + 2, msl],
                T.E1T[:, 2 * h : 2 * h + 2, nsl],
                start=(h == 0), stop=(h == 1), perf_mode=DR,
            )
        o4 = P.ost.tile([128, 512], F32, tag="o4", bufs=5, name=f"o4_{b}_{md}_{ni}")
        nc.vector.tensor_mul(o4, bps, crn[:, md, :])
        nc.sync.dma_start(
            out=Od[b, 3 * D + md * 128 : 3 * D + (md + 1) * 128, nsl], in_=o4
        )


def build(bl=BL, num_devices=NCORES, enable_asserts=False):
    from contextlib import ExitStack

    nc = bacc.Bacc(
        "TRN2",
        target_bir_lowering=False,
        debug=False,
        enable_asserts=enable_asserts,
        num_devices=num_devices,
    )
    Cd = nc.dram_tensor("C", (bl, D, LC), F32, kind="ExternalInput").ap()
    Qd = nc.dram_tensor("Q", (bl, D, LQ), F32, kind="ExternalInput").ap()
    wd = nc.dram_tensor("w", (3 * D,), F32, kind="ExternalInput").ap()
    Od = nc.dram_tensor("out", (bl, 4 * D, LC), F32, kind="ExternalOutput").ap()

    with tile.TileContext(nc) as tc, ExitStack() as ctx:
        P = _pools(tc, ctx)
        K = Ctx()
        # w first (tiny; Qmod/c2 depend on it), then example 0's input
        # DMAs, so everything overlaps const setup / engine bring-up
        K.wsb = P.const.tile([128, 12], F32, name="wsb")
        nc.sync.dma_start(out=K.wsb, in_=wd.rearrange("(c p) -> p c", p=128))
        K.ident = P.const.tile([128, 128], F32, name="ident")
        make_identity(nc, K.ident)
        tiles, pend = {}, {}
        tiles[0] = Ctx()
        _phase_A_loads(nc, P, tiles[0], Cd, Qd, Od, 0)

        K.wsbR = P.const.tile([128, 12], BF16, name="wsbR")
        nc.vector.tensor_copy(K.wsbR, K.wsb)
        K.identR = P.const.tile([128, 128], BF16, name="identR")
        nc.vector.tensor_copy(K.identR, K.ident)
        K.identF8 = P.const.tile([128, 128], F8, name="identF8")
        nc.vector.tensor_copy(K.identF8, K.ident)
        ones_col_f = P.const.tile([128, 1], F32, name="ocf")
        nc.vector.memset(ones_col_f, 1.0)
        K.ones_col = P.const.tile([128, 1], F8, name="oc")
        nc.vector.tensor_copy(K.ones_col, ones_col_f)
        K.ones_matb = P.const.tile([128, 128], BF16, name="omb")
        nc.vector.memset(K.ones_matb, 1.0)

        def _big_tiles(T, b):
            T.E1T = P.big.tile(
                [128, KJ, LC], F8, tag="e1t", bufs=2, name=f"e1t{b}"
            )
            T.Qbt = P.big.tile(
                [128, KJ, D], F8, tag="qbt", bufs=2, name=f"qbt{b}"
            )
            T.T2s = P.big.tile([128, KJ, D], F8, tag="t2s", name=f"t2s{b}")

        tail = []
        for b in range(bl):
            T = tiles[b]
            last = b + 1 >= bl
            if b == 0:
                _big_tiles(T, b)
                _phase_A_body(nc, P, K, T, b)
            inject = {0: (lambda bb=b: _o1_writes(nc, tiles[bb], Od, bb))}
            if not last:
                tiles[b + 1] = Ctx()
                inject[1] = (
                    lambda bb=b + 1: _phase_A_loads(nc, P, tiles[bb], Cd, Qd, Od, bb)
                )
            # previous example's deferred output phases ride this CD loop
            for s, fn in enumerate(tail):
                prev = inject.get(s)
                inject[s] = (
                    (lambda p=prev, f=fn: (p(), f())) if prev else fn
                )
            tail = _phase_CD(nc, P, K, T, Od, b, inject=inject)
            if not last:
                _big_tiles(tiles[b + 1], b + 1)
                _phase_A_body(nc, P, K, tiles[b + 1], b + 1)
        for fn in tail:
            fn()
    nc.compile()
    return nc


_NC = None


def kernel(C, Q, cmask, qmask, w):
    global _NC
    C = np.ascontiguousarray(np.asarray(C, dtype=np.float32))
    Q = np.ascontiguousarray(np.asarray(Q, dtype=np.float32))
    w = np.ascontiguousarray(np.asarray(w, dtype=np.float32))
    # masks are all-ones per the problem spec; softmax masking is a no-op
    if _NC is None:
        _NC = build()
    in_maps = [
        {
            "C": np.ascontiguousarray(C[i * BL : (i + 1) * BL]),
            "Q": np.ascontiguousarray(Q[i * BL : (i + 1) * BL]),
            "w": w,
        }
        for i in range(NCORES)
    ]
    res = run_bass_kernel_spmd(_NC, in_maps, core_ids=list(range(NCORES)))
    return np.concatenate([res.results[i]["out"] for i in range(NCORES)], axis=0)
